# revision 12
# baseline (speedup 1.0000x reference)
"""ChromosomeEmbedding kernel for 8x Trainium2 NeuronCores.

Computes out[b, j, d] = ce[chr[b]-1, d] for b in [0,512), j in [0,2001),
d in [0,128). Data-parallel: the batch is sharded 64 samples/core across
8 cores; the 24x128 table lookup (64 rows -> 32 KB) is folded into host
input prep, so the device program is a pure HBM-write streamer. The
per-core output shard is 65.5 MB.

Key HW facts (measured via per-descriptor NTFF records):
  * A dma_start whose DRAM-side outer extent is 128 (matching the 128
    SBUF partitions 1:1) spreads descriptors evenly over all 16 SDMA
    engines and each 32 KB descriptor then moves at the ~27 GB/s SBUF
    AXI port line rate (~400 GB/s/core aggregate). A 64-partition DMA
    instead lands engines on foreign ports (~18 GB/s each), and a small
    DRAM outer extent collapses the whole DMA onto 1-2 engines.
  * All engines leave the framework's iteration-start barrier together
    (~6.9 us), and the SBUF seed load costs a further ~3 us of
    transfer + completion receipt before SBUF-sourced output DMAs can
    start. DRAM->DRAM opening rungs hide that window.

Layout: SBUF partition p (and host-side pre row p) holds the embedding
row of sample p//2 -- each sample duplicated on an adjacent partition
pair, replicated along REP bin-columns. Every output DMA is then a
128-partition "dual-block" transfer writing bins [a, a+w) from even
copies plus [a+w, a+2w) from odd copies via a (64, 2, w, 128) DRAM-side
access pattern whose outer dims walk partitions 0..127 in order.

Per-core device program (identical SPMD program on all cores, raw bacc):
  1. At barrier-exit, each ring first issues half the seed load
     (rep[:, 0:8] <- pre[:, 0:8], isem) -- split so each queue's first
     packet is small, since engines switch queues only at packet
     boundaries -- then a DRAM->DRAM rung writing its first 32 bins
     straight from pre (1 MB): output bytes flow from ~8.5 us with no
     SBUF dependency while the seed receipts land.
  2. Three doubling copies on the vector engine extend 8 -> 64 columns.
  3. SBUF ladder per ring (w = 8, 16, 32, 64 half-width, gated on isem /
     the replica width available), then steady 128-bin 4 MB DMAs. Sync
     walks bins [0, SPLIT), scalar [SPLIT, 2001); the final remainder
     DMA overlaps one already-written bin when the range is odd
     (identical bytes, so order doesn't matter).
  4. Minimal tail: each ring waits for its completion count, bumps a
     done-sem; gpsimd resets DMA state and clears sems for re-execution.
"""

import functools

import numpy as np

from concourse import bacc, mybir
from concourse.bass_utils import run_bass_kernel_spmd

N_CORES = 8
BS = 512
BPC = BS // N_CORES  # 64 samples per core
NBIN = 2001
DIM = 128
N_CHR = 24
REP = 64  # replicated copies of each row held in SBUF (per partition)
W0 = 16  # host-side pre-replication width (bins); DRAM rungs use all 16
SEED = 8  # pre columns loaded into SBUF as the replication seed
SPLIT = 1021  # bins walked by the sync ring; scalar ring takes the rest
# (sync gets the extra bins: the scalar queue also carries the seed load
# and measured slightly slower per-queue NTFF rates on an even split)
F32 = mybir.dt.float32


def _need_v(w):
    """Doubling-copy count required before rep[:, 0:w] is valid."""
    v = 0
    have = SEED
    while have < w:
        have *= 2
        v += 1
    return v


def _ring_plan(lo, hi):
    """Cover bins [lo, hi) with dual-block DMAs: list of (kind, out_off,
    w, v) where the DMA writes [out_off, out_off+2w). kind "dram"
    sources pre_h[:, 0:w] (no SBUF dependency), "sbuf" sources
    rep[:, 0:w] (needs isem, and v doubling copies). If the range is
    odd, the final DMA overlaps one bin already covered (same bytes)."""
    plan = []
    off = lo
    if hi - off >= 2 * W0:
        plan.append(("dram", off, W0, 0))
        off += 2 * W0
    for w in (SEED, 16, 32):
        if hi - off < 2 * w:
            break
        plan.append(("sbuf", off, w, _need_v(w)))
        off += 2 * w
    while hi - off >= 2 * REP:
        plan.append(("sbuf", off, REP, _need_v(REP)))
        off += 2 * REP
    r = hi - off
    if r > 0:
        w = (r + 1) // 2  # covers 2w >= r bins, overlapping (2w - r) bins
        plan.append(("sbuf", hi - 2 * w, w, _need_v(w)))
    return plan


@functools.lru_cache(maxsize=1)
def build_nc():
    nc = bacc.Bacc("TRN2", target_bir_lowering=False)

    pre_h = nc.declare_dram_parameter("pre", [128, W0, DIM], F32, isOutput=False)
    out_h = nc.declare_dram_parameter("out", [BPC, NBIN, DIM], F32, isOutput=True)

    with (
        nc.sbuf_tensor("rep", [128, REP, DIM], F32) as rep,
        nc.semaphore("ssem") as ssem,  # sync-ring DMA completions
        nc.semaphore("asem") as asem,  # scalar-ring DMA completions
        nc.semaphore("isem") as isem,  # seed-load completion
        nc.semaphore("vsem") as vsem,  # doubling-copy completions
        nc.semaphore("done") as done,  # ring-drained markers
    ):
        sync_plan = _ring_plan(0, SPLIT)
        scal_plan = _ring_plan(SPLIT, NBIN)

        def dual_out(off, w):
            """(64, 2, w, DIM) view of out_h[:, off:off+2w, :]: outer
            index p = 2b + r writes sample b, bins [off+r*w,
            off+(r+1)*w) -- outer extent 128 keeps the engine spread."""
            return out_h[:, off : off + 2 * w, :].rearrange(
                "b (r w) d -> b r w d", r=2
            )

        # Seed load split across both queues so each ring's first packet
        # is small (engines switch queues only at packet boundaries; a
        # big first packet on one queue starves the other for ~4 us).
        # Each ring then streams its DRAM rung while the receipts land.
        half = SEED // 2
        nc.sync.dma_start(out=rep[:, 0:half, :], in_=pre_h[:, 0:half, :]).then_inc(
            isem, 16
        )
        nc.scalar.dma_start(
            out=rep[:, half:SEED, :], in_=pre_h[:, half:SEED, :]
        ).then_inc(isem, 16)

        # Vector engine: doubling replication SEED -> REP columns.
        nc.vector.wait_ge(isem, 32)
        w = SEED
        while w < REP:
            nc.vector.tensor_copy(
                out=rep[:, w : 2 * w, :], in_=rep[:, 0:w, :]
            ).then_inc(vsem, 1)
            w *= 2

        def run_ring(eng, plan, own_sem):
            waited_i = False
            seen_v = 0
            for kind, off, w, v in plan:
                if kind == "sbuf" and not waited_i:
                    eng.wait_ge(isem, 32)
                    waited_i = True
                if v > seen_v:
                    eng.wait_ge(vsem, v)
                    seen_v = v
                src = pre_h[:, 0:w, :] if kind == "dram" else rep[:, 0:w, :]
                eng.dma_start(out=dual_out(off, w), in_=src).then_inc(own_sem, 16)

        run_ring(nc.sync, sync_plan, ssem)
        run_ring(nc.scalar, scal_plan, asem)

        # Tail: wait for both rings to drain, then restore sem state so
        # the NEFF can be re-executed (sems are only load-time zeroed).
        nc.sync.wait_ge(ssem, 16 * len(sync_plan))
        nc.sync.sem_inc(done, 1)
        nc.scalar.wait_ge(asem, 16 * len(scal_plan))
        nc.scalar.sem_inc(done, 1)

        nc.gpsimd.wait_ge(done, 2)
        nums = sorted(s.num for s in (ssem, asem, isem, vsem, done))
        lo, hi = nums[0], nums[-1]
        if nums == list(range(lo, hi + 1)):
            ranges = [range(lo, hi + 1)]
        else:
            ranges = [range(n, n + 1) for n in nums]
        for r in ranges:
            nc.gpsimd.dma_reset(r)
            nc.gpsimd.sem_clear(r)

    nc.compile()
    return nc


def make_in_maps(chr_full: np.ndarray, ce: np.ndarray):
    ce_f32 = np.asarray(ce, dtype=np.float32)
    idx = np.asarray(chr_full).astype(np.int64) - 1
    maps = []
    for c in range(N_CORES):
        rows = ce_f32[idx[c * BPC : (c + 1) * BPC]]  # [64, 128]
        # Sample-interleaved duplicate: row p holds sample p//2, so
        # partition p maps to DRAM outer indices (b=p//2, r=p%2).
        both = np.repeat(rows, 2, axis=0)  # [128, 128]
        pre = np.repeat(both[:, None, :], W0, axis=1)  # [128, W0, 128]
        maps.append({"pre": np.ascontiguousarray(pre)})
    return maps


def kernel(tensor=None, chr=None, ce=None, **_unused):
    chr_np = np.asarray(chr)
    ce_np = np.asarray(ce)
    nc = build_nc()
    res = run_bass_kernel_spmd(
        nc, make_in_maps(chr_np, ce_np), core_ids=list(range(N_CORES))
    )
    out = np.concatenate([r["out"] for r in res.results], axis=0)
    return out.astype(np.float32)
